# revision 33
# baseline (speedup 1.0000x reference)
"""Trainium2 Bass kernel for nn_GeneSetPlaceholderAggregator.

Computes out[b,s,d] = sum_g x[b,g,d] * W[s,g]  (einsum 'bgd,sg->bsd')
with B=64, G=20000, D=16, S=128.

Strategy:
- Shard the contraction axis G across 8 cores (2500 genes each, zero-padded
  to 2560 = 20 chunks of 128).  Each core computes a full partial output
  [S=128, B*D=1024] via PSUM-accumulated matmuls (contraction on the
  partition dim); the host sums the 8 partials.
- Mixed precision to cut HBM traffic (the sole bottleneck): W fp16, the
  first 12 x-chunks fp16, the last 8 x-chunks fp8e4 (HW matmul with fp16
  lhsT x fp8 rhs verified exact).  Measured end-to-end rel error ~1.66e-2
  against the fp32 reference, under the 2e-2 gate; fp16-only is ~3.6e-4.
- W rides in the first DMA group (prepended columns) so there is one fewer
  dma_start; trailing DMA groups are kept few and large because the last
  groups' completion semaphores serialize ~1.3us each behind a straggler
  SDMA engine once the bulk stream drains.
- Warm-up matmuls on a zeroed tile during the first-DMA latency window trip
  the HAM activity ramp; post-body keep-alive matmuls (reading the output
  tile, so they cannot delay real work) hold the clock up through the fixed
  ~250-instruction semaphore-teardown epilogue while the output-DMA receipt
  is pending.
- Output returned as fp16 [S, B*D], one fused DMA (host sums partials in
  fp32).
"""

import os

import numpy as np
import ml_dtypes

import concourse.mybir as mybir
from concourse import bass
from concourse.bacc import Bacc
from concourse.bass_utils import run_bass_kernel_spmd
from concourse.tile import TileContext

B, G, D, S = 64, 20000, 16, 128
N_CORES = 8
K = 128                        # contraction tile = partition dim
N_CHUNKS = 20                  # chunks per core
N_C16 = int(os.environ.get("KRN_C16", "12"))   # leading chunks kept in fp16
N_C8 = N_CHUNKS - N_C16        # trailing chunks in fp8e4
G_LOC = K * N_CHUNKS           # 2560 genes per core (padded)
G_PAD = G_LOC * N_CORES        # 20480
BD = B * D                     # 1024
FREE = 512                     # max fp32 free dim per PSUM bank
N_FREE = BD // FREE            # 2
W_COLS = N_CHUNKS * S          # 2560 W columns, prepended to group 0
# DMA groups in ring (arrival) order: (dtype-kind, chunk indices).
# Group 0 carries W and is small so the PE starts early; fp8 chunks ride
# mid-stream; a small fp16 group lands last so the end-of-stream straggler
# crawl and the PE backlog stay short.  PSUM accumulation follows arrival
# order (commutative).
_ORDERS = {
    (12, 8): [
        ("f16", [0, 1], "sync"),
        ("f8", [16, 17, 18, 19], "scalar"),
        ("f16", [2, 3, 4], "sync"),
        ("f16", [5, 6, 7, 8], "sync"),
        ("f8", [12, 13, 14, 15], "sync"),
        ("f16", [9, 10, 11], "sync"),
    ],
    (20, 0): [
        ("f16", [0, 1], "sync"),
        ("f16", [2, 3, 4], "sync"),
        ("f16", [5, 6, 7, 8], "sync"),
        ("f16", [9, 10, 11, 12], "sync"),
        ("f16", [13, 14, 15, 16], "sync"),
        ("f16", [17, 18, 19], "sync"),
    ],
}
ORDER = _ORDERS[(N_C16, N_C8)]
N_WARM = 24                    # PE-ramp matmuls during first-DMA latency
N_KEEP = 30                    # post-body matmuls: hold clocks up into teardown

FP16 = mybir.dt.float16
FP8 = mybir.dt.float8e4
NP_FP8 = ml_dtypes.float8_e4m3


def build_nc() -> bass.Bass:
    nc = Bacc("TRN2", target_bir_lowering=False)

    x16_d = nc.declare_dram_parameter(
        "x16", [K, W_COLS + N_C16 * BD], FP16, isOutput=False
    )
    x8_d = (
        nc.declare_dram_parameter("x8", [K, N_C8 * BD], FP8, isOutput=False)
        if N_C8
        else None
    )
    out = nc.declare_dram_parameter("out", [S, BD], FP16, isOutput=True)

    with TileContext(nc) as tc:
        with (
            tc.tile_pool(name="gp", bufs=1) as gp,
            tc.tile_pool(name="op", bufs=1) as op,
            tc.tile_pool(name="ps", bufs=1, space="PSUM") as ps,
        ):
            psums = [
                ps.tile([S, FREE], mybir.dt.float32, name=f"psum{j}")
                for j in range(N_FREE)
            ]
            warm_ps = ps.tile([K, K], mybir.dt.float32, name="warm_ps")
            warm = gp.tile([K, K], FP16, name="warm", tag="warm")
            nc.vector.memset(warm[:], 0.0)
            for _ in range(N_WARM):
                nc.tensor.matmul(
                    warm_ps[:], lhsT=warm[:], rhs=warm[:], start=True, stop=True
                )

            # x16_d columns: [W | fp16 chunks in arrival order]; x8_d: fp8
            # chunks in arrival order.  _shard_inputs packs to match ORDER.
            rhs_of = {}            # chunk -> (tile, col offset)
            w_t = None             # tile holding W (group 0), offset 0
            seq = []               # chunks in arrival order
            for g, (kind, chunks, ring) in enumerate(ORDER):
                sz = len(chunks)
                wc = W_COLS if g == 0 else 0
                eng = nc.sync if ring == "sync" else nc.scalar
                assert chunks == list(range(chunks[0], chunks[0] + sz))
                if kind == "f16":
                    o16 = chunks[0]          # fp16 chunks are 0..N_C16-1
                    g_t = gp.tile([K, wc + sz * BD], FP16, name=f"g{g}",
                                  tag=f"g{g}")
                    eng.dma_start(
                        out=g_t[:],
                        in_=x16_d[:, W_COLS - wc + o16 * BD:
                                  W_COLS + (o16 + sz) * BD],
                    )
                else:
                    o8 = chunks[0] - N_C16   # fp8 chunks are N_C16..19
                    g_t = gp.tile([K, sz * BD], FP8, name=f"g{g}", tag=f"g{g}")
                    eng.dma_start(
                        out=g_t[:], in_=x8_d[:, o8 * BD:(o8 + sz) * BD]
                    )
                if g == 0:
                    w_t = g_t
                for l, c in enumerate(chunks):
                    rhs_of[c] = (g_t, wc + l * BD)
                    seq.append(c)

            for i, c in enumerate(seq):
                t, base = rhs_of[c]
                for j in range(N_FREE):
                    nc.tensor.matmul(
                        psums[j][:],
                        lhsT=w_t[:, c * S:(c + 1) * S],
                        rhs=t[:, base + j * FREE:base + (j + 1) * FREE],
                        start=(i == 0),
                        stop=(i == N_CHUNKS - 1),
                    )

            o_t = op.tile([S, BD], FP16)
            nc.vector.tensor_copy(out=o_t[:, :FREE], in_=psums[0][:])
            nc.scalar.copy(out=o_t[:, FREE:], in_=psums[1][:])
            nc.sync.dma_start(out=out[:, :], in_=o_t[:])

            # Fill the PE-idle gap while the output DMA receipt is pending so
            # the activity monitor doesn't down-clock the teardown epilogue.
            # Depends on o_t (output copies), so it cannot delay real work.
            for _ in range(N_KEEP):
                nc.tensor.matmul(
                    warm_ps[:], lhsT=o_t[:, :K], rhs=o_t[:, :K],
                    start=True, stop=True,
                )
    nc.compile()
    return nc


_CACHE: dict = {}


def _get_nc() -> bass.Bass:
    if "nc" not in _CACHE:
        _CACHE["nc"] = build_nc()
    return _CACHE["nc"]


def _shard_inputs(x: np.ndarray, W: np.ndarray) -> list[dict[str, np.ndarray]]:
    # Gene-major layouts, partition-major per core:
    #   XG [G_PAD, BD]  (gene-major x),  WG [G_PAD, S]  (gene-major W)
    #   per core: chunk c, partition p  <-  gene i*G_LOC + c*K + p
    XG = np.zeros((G_PAD, BD), dtype=np.float32)
    XG[:G] = x.transpose(1, 0, 2).reshape(G, BD)
    WG = np.zeros((G_PAD, S), dtype=np.float16)
    WG[:G] = W.T.astype(np.float16)

    XGc = XG.reshape(N_CORES, N_CHUNKS, K, BD).transpose(0, 2, 1, 3)
    WGc = np.ascontiguousarray(
        WG.reshape(N_CORES, N_CHUNKS, K, S).transpose(0, 2, 1, 3)
    ).reshape(N_CORES, K, W_COLS)
    X16 = np.ascontiguousarray(XGc[:, :, :N_C16]).astype(np.float16).reshape(
        N_CORES, K, N_C16 * BD
    )
    X16W = np.concatenate([WGc, X16], axis=2)
    maps = [{"x16": X16W[i]} for i in range(N_CORES)]
    if N_C8:
        X8 = np.ascontiguousarray(XGc[:, :, N_C16:]).astype(NP_FP8).reshape(
            N_CORES, K, N_C8 * BD
        )
        for i in range(N_CORES):
            maps[i]["x8"] = X8[i]
    return maps


def run(x: np.ndarray, W: np.ndarray, **spmd_kwargs):
    nc = _get_nc()
    in_maps = _shard_inputs(x, W)
    res = run_bass_kernel_spmd(nc, in_maps, list(range(N_CORES)), **spmd_kwargs)
    partial = np.zeros((S, BD), dtype=np.float32)
    for r in res.results:
        partial += r["out"].astype(np.float32)
    out = partial.reshape(S, B, D).transpose(1, 0, 2)
    return np.ascontiguousarray(out), res


def kernel(x: np.ndarray, W: np.ndarray) -> np.ndarray:
    out, _ = run(x, W)
    return out


# revision 34
# speedup vs baseline: 1.0308x; 1.0308x over previous
"""Trainium2 Bass kernel for nn_GeneSetPlaceholderAggregator.

Computes out[b,s,d] = sum_g x[b,g,d] * W[s,g]  (einsum 'bgd,sg->bsd')
with B=64, G=20000, D=16, S=128.

Strategy:
- Shard the contraction axis G across 8 cores (2500 genes each, zero-padded
  to 2560 = 20 chunks of 128).  Each core computes a full partial output
  [S=128, B*D=1024] via PSUM-accumulated matmuls (contraction on the
  partition dim); the host sums the 8 partials.
- Mixed precision to cut HBM traffic (the sole bottleneck): W fp16, the
  first 12 x-chunks fp16, the last 8 x-chunks fp8e4 (HW matmul with fp16
  lhsT x fp8 rhs verified exact).  Measured end-to-end rel error ~1.66e-2
  against the fp32 reference, under the 2e-2 gate; fp16-only is ~3.6e-4.
- W rides in the first DMA group (prepended columns) so there is one fewer
  dma_start; trailing DMA groups are kept few and large because the last
  groups' completion semaphores serialize ~1.3us each behind a straggler
  SDMA engine once the bulk stream drains.
- Warm-up matmuls on a zeroed tile during the first-DMA latency window trip
  the HAM activity ramp; post-body keep-alive matmuls (reading the output
  tile, so they cannot delay real work) hold the clock up through the fixed
  ~250-instruction semaphore-teardown epilogue while the output-DMA receipt
  is pending.
- Output returned as fp16 [S, B*D], one fused DMA (host sums partials in
  fp32).
"""

import os

import numpy as np
import ml_dtypes

import concourse.mybir as mybir
from concourse import bass
from concourse.bacc import Bacc
from concourse.bass_utils import run_bass_kernel_spmd
from concourse.tile import TileContext

B, G, D, S = 64, 20000, 16, 128
N_CORES = 8
K = 128                        # contraction tile = partition dim
N_CHUNKS = 20                  # chunks per core
N_C16 = int(os.environ.get("KRN_C16", "12"))   # leading chunks kept in fp16
N_C8 = N_CHUNKS - N_C16        # trailing chunks in fp8e4
G_LOC = K * N_CHUNKS           # 2560 genes per core (padded)
G_PAD = G_LOC * N_CORES        # 20480
BD = B * D                     # 1024
FREE = 512                     # max fp32 free dim per PSUM bank
N_FREE = BD // FREE            # 2
W_COLS = N_CHUNKS * S          # 2560 W columns, prepended to group 0
# DMA groups in ring (arrival) order: (dtype-kind, chunk indices).
# Group 0 carries W and is small so the PE starts early; fp8 chunks ride
# mid-stream; a small fp16 group lands last so the end-of-stream straggler
# crawl and the PE backlog stay short.  PSUM accumulation follows arrival
# order (commutative).
_ORDERS = {
    (12, 8): [
        ("f16", [0, 1], "sync"),
        ("f16", [2, 3, 4], "sync"),
        ("f16", [5, 6, 7, 8], "sync"),
        ("f8", [12, 13, 14, 15], "sync"),
        ("f8", [16, 17, 18, 19], "sync"),
        ("f16", [9, 10, 11], "sync"),
    ],
    (20, 0): [
        ("f16", [0, 1], "sync"),
        ("f16", [2, 3, 4], "sync"),
        ("f16", [5, 6, 7, 8], "sync"),
        ("f16", [9, 10, 11, 12], "sync"),
        ("f16", [13, 14, 15, 16], "sync"),
        ("f16", [17, 18, 19], "sync"),
    ],
}
ORDER = _ORDERS[(N_C16, N_C8)]
N_WARM = 24                    # PE-ramp matmuls during first-DMA latency
N_KEEP = 30                    # post-body matmuls: hold clocks up into teardown

FP16 = mybir.dt.float16
FP8 = mybir.dt.float8e4
NP_FP8 = ml_dtypes.float8_e4m3


def build_nc() -> bass.Bass:
    nc = Bacc("TRN2", target_bir_lowering=False)

    x16_d = nc.declare_dram_parameter(
        "x16", [K, W_COLS + N_C16 * BD], FP16, isOutput=False
    )
    x8_d = (
        nc.declare_dram_parameter("x8", [K, N_C8 * BD], FP8, isOutput=False)
        if N_C8
        else None
    )
    out = nc.declare_dram_parameter("out", [S, BD], FP16, isOutput=True)

    with TileContext(nc) as tc:
        with (
            tc.tile_pool(name="gp", bufs=1) as gp,
            tc.tile_pool(name="op", bufs=1) as op,
            tc.tile_pool(name="ps", bufs=1, space="PSUM") as ps,
        ):
            psums = [
                ps.tile([S, FREE], mybir.dt.float32, name=f"psum{j}")
                for j in range(N_FREE)
            ]
            warm_ps = ps.tile([K, K], mybir.dt.float32, name="warm_ps")
            warm = gp.tile([K, K], FP16, name="warm", tag="warm")
            nc.vector.memset(warm[:], 0.0)
            for _ in range(N_WARM):
                nc.tensor.matmul(
                    warm_ps[:], lhsT=warm[:], rhs=warm[:], start=True, stop=True
                )

            # x16_d columns: [W | fp16 chunks in arrival order]; x8_d: fp8
            # chunks in arrival order.  _shard_inputs packs to match ORDER.
            rhs_of = {}            # chunk -> (tile, col offset)
            w_t = None             # tile holding W (group 0), offset 0
            seq = []               # chunks in arrival order
            for g, (kind, chunks, ring) in enumerate(ORDER):
                sz = len(chunks)
                wc = W_COLS if g == 0 else 0
                eng = nc.sync if ring == "sync" else nc.scalar
                assert chunks == list(range(chunks[0], chunks[0] + sz))
                if kind == "f16":
                    o16 = chunks[0]          # fp16 chunks are 0..N_C16-1
                    g_t = gp.tile([K, wc + sz * BD], FP16, name=f"g{g}",
                                  tag=f"g{g}")
                    eng.dma_start(
                        out=g_t[:],
                        in_=x16_d[:, W_COLS - wc + o16 * BD:
                                  W_COLS + (o16 + sz) * BD],
                    )
                else:
                    o8 = chunks[0] - N_C16   # fp8 chunks are N_C16..19
                    g_t = gp.tile([K, sz * BD], FP8, name=f"g{g}", tag=f"g{g}")
                    eng.dma_start(
                        out=g_t[:], in_=x8_d[:, o8 * BD:(o8 + sz) * BD]
                    )
                if g == 0:
                    w_t = g_t
                for l, c in enumerate(chunks):
                    rhs_of[c] = (g_t, wc + l * BD)
                    seq.append(c)

            for i, c in enumerate(seq):
                t, base = rhs_of[c]
                for j in range(N_FREE):
                    nc.tensor.matmul(
                        psums[j][:],
                        lhsT=w_t[:, c * S:(c + 1) * S],
                        rhs=t[:, base + j * FREE:base + (j + 1) * FREE],
                        start=(i == 0),
                        stop=(i == N_CHUNKS - 1),
                    )

            o_t = op.tile([S, BD], FP16)
            nc.vector.tensor_copy(out=o_t[:, :FREE], in_=psums[0][:])
            nc.scalar.copy(out=o_t[:, FREE:], in_=psums[1][:])
            nc.sync.dma_start(out=out[:, :], in_=o_t[:])

            # Fill the PE-idle gap while the output DMA receipt is pending so
            # the activity monitor doesn't down-clock the teardown epilogue.
            # Depends on o_t (output copies), so it cannot delay real work.
            for _ in range(N_KEEP):
                nc.tensor.matmul(
                    warm_ps[:], lhsT=o_t[:, :K], rhs=o_t[:, :K],
                    start=True, stop=True,
                )
    nc.compile()
    return nc


_CACHE: dict = {}


def _get_nc() -> bass.Bass:
    if "nc" not in _CACHE:
        _CACHE["nc"] = build_nc()
    return _CACHE["nc"]


def _shard_inputs(x: np.ndarray, W: np.ndarray) -> list[dict[str, np.ndarray]]:
    # Gene-major layouts, partition-major per core:
    #   XG [G_PAD, BD]  (gene-major x),  WG [G_PAD, S]  (gene-major W)
    #   per core: chunk c, partition p  <-  gene i*G_LOC + c*K + p
    XG = np.zeros((G_PAD, BD), dtype=np.float32)
    XG[:G] = x.transpose(1, 0, 2).reshape(G, BD)
    WG = np.zeros((G_PAD, S), dtype=np.float16)
    WG[:G] = W.T.astype(np.float16)

    XGc = XG.reshape(N_CORES, N_CHUNKS, K, BD).transpose(0, 2, 1, 3)
    WGc = np.ascontiguousarray(
        WG.reshape(N_CORES, N_CHUNKS, K, S).transpose(0, 2, 1, 3)
    ).reshape(N_CORES, K, W_COLS)
    X16 = np.ascontiguousarray(XGc[:, :, :N_C16]).astype(np.float16).reshape(
        N_CORES, K, N_C16 * BD
    )
    X16W = np.concatenate([WGc, X16], axis=2)
    maps = [{"x16": X16W[i]} for i in range(N_CORES)]
    if N_C8:
        X8 = np.ascontiguousarray(XGc[:, :, N_C16:]).astype(NP_FP8).reshape(
            N_CORES, K, N_C8 * BD
        )
        for i in range(N_CORES):
            maps[i]["x8"] = X8[i]
    return maps


def run(x: np.ndarray, W: np.ndarray, **spmd_kwargs):
    nc = _get_nc()
    in_maps = _shard_inputs(x, W)
    res = run_bass_kernel_spmd(nc, in_maps, list(range(N_CORES)), **spmd_kwargs)
    partial = np.zeros((S, BD), dtype=np.float32)
    for r in res.results:
        partial += r["out"].astype(np.float32)
    out = partial.reshape(S, B, D).transpose(1, 0, 2)
    return np.ascontiguousarray(out), res


def kernel(x: np.ndarray, W: np.ndarray) -> np.ndarray:
    out, _ = run(x, W)
    return out
